# revision 5
# baseline (speedup 1.0000x reference)
"""EventSpecificTimingHeads Trainium2 kernel v2 (8 NeuronCores, SPMD).

Shards E=16 events across 8 cores (2 events/core). Per core, for each
(event, batch) the pipeline is:

  scores^T[j,i] = k_j . q_i  (q pre-scaled; 16 matmuls N=512)
  P^T = exp(scores^T)        split between ACT (table exp) and DVE
                             (Schraudolph bf16 bit-trick: P_bits =
                             round(s*128/ln2 + 16241.5) as int16 -> bf16)
  ctx_u[i,(h,d|l)] = P^T as lhsT @ [v|1]   (N=33 matmuls, PSUM-accumulated
                             over j; row sums l land in column 32)
  ctxn = ctx_u * (1/l)       DVE (fused with the PSUM->SBUF copy)
  ctxT = xbar-DMA transpose of ctxn  (SBUF->SBUF, frees PE + DVE)
  gp = Wf @ ctxT             Wf = W1 @ Wo fused on host (one matmul)
  h1 = relu(gp + c1) on ACT  (c1 = W1(Wo bv + bo) + b1, host-folded)
  logits columns: per i-chunk, h1_aug as lhsT @ w2_aug -> [128, 1] psum
                             (N=1 matmuls into persistent accumulator)
  end: transpose the [128, 64] logit accumulator once, DMA out.
"""
import sys

if "/opt/trn_rl_repo" not in sys.path:
    sys.path.insert(0, "/opt/trn_rl_repo")

import numpy as np
import ml_dtypes

import concourse.bass as bass
import concourse.bacc as bacc
import concourse.tile as tile
from concourse import mybir
from concourse import masks
from concourse.bass_utils import run_bass_kernel_spmd

BF16 = mybir.dt.bfloat16
F32 = mybir.dt.float32
I16 = mybir.dt.int16
AF = mybir.ActivationFunctionType
ALU = mybir.AluOpType

E, D, B, S, H, Dh, H2 = 16, 128, 8, 512, 4, 32, 64
T = B * S            # 4096
EV = 2               # events per core
NCORES = 8

# Schraudolph-for-bf16: bits16 = s * 2^7/ln2 + (127*2^7 - C); C tuned for
# softmax error, robust to round-vs-floor conversion.
SCH_SCALE = 128.0 / float(np.log(2.0))
SCH_BASE = 16256.0 - 14.5

_CACHED_NC = None


def is_act_chunk(eb, idx):
    """exp engine split: ~62% of chunks on ACT (relu lives on DVE now).
    evb0 is DVE-heavy since ACT does the startup q/k bias copies."""
    if eb == 0:
        return idx in (2, 5, 7)
    if idx % 2 == 0:
        return True
    return idx == 7 if eb % 2 == 0 else idx == 5


def build_nc():
    nc = bacc.Bacc(None, target_bir_lowering=False, debug=False)

    xT_d = nc.declare_dram_parameter("xT", [D, T], BF16, isOutput=False)
    wqkvT_d = nc.declare_dram_parameter("wqkvT", [D, EV, 3, D], BF16, isOutput=False)
    bqk_d = nc.declare_dram_parameter("bqk", [D, EV, 2], F32, isOutput=False)
    wfT_d = nc.declare_dram_parameter("wfT", [D, EV, H2], BF16, isOutput=False)
    c1b_d = nc.declare_dram_parameter("c1b", [H2, EV], F32, isOutput=False)
    w2a_d = nc.declare_dram_parameter("w2a", [H2 + 1, EV], BF16, isOutput=False)
    out_d = nc.declare_dram_parameter("out", [EV, B, S], F32, isOutput=True)

    with tile.TileContext(nc) as tc:
        with (
            tc.tile_pool(name="single", bufs=1) as single,
            tc.tile_pool(name="work", bufs=2) as work,
            tc.tile_pool(name="stp", bufs=3, space="PSUM") as stp,
            tc.tile_pool(name="gpp", bufs=1, space="PSUM") as gpp,
            tc.tile_pool(name="ctp", bufs=1, space="PSUM") as ctp,
        ):
            # ---- resident SBUF tensors ----
            xT_sb = single.tile([D, T], BF16)
            wqkvT_sb = single.tile([D, EV, 3, D], BF16)
            bqk_sb = single.tile([D, EV, 2], F32)
            wfT_sb = single.tile([D, EV, H2], BF16)
            c1b_sb = single.tile([H2, EV], F32)
            w2a_sb = single.tile([H2 + 1, EV], BF16)
            ident = single.tile([D, D], BF16)
            qT_sb = single.tile([D, EV, T], BF16)
            kT_sb = single.tile([D, EV, T], BF16)
            # v_aug: [j-in-chunk, ev, b, jc, h, 33]; col 32 of each h = 1.0
            v_sb = single.tile([D, EV, B, 4, H, Dh + 1], BF16)
            h1a_sb = single.tile([H2 + 1, 2, S], BF16)  # row 64 = ones
            lg_sb = single.tile([D, 64], BF16)
            lgT_sb = single.tile([64, D], F32)

            # one persistent PSUM bank: ctx accumulation region (reused by
            # every PV half) + the 64 logit columns
            cxl = ctp.tile([D, 2 * H * (Dh + 1) + 64], F32, name="cxl", tag="lg")
            ctxps = cxl[:, 0:2 * H * (Dh + 1)].rearrange(
                "p (c h d) -> p c h d", c=2, h=H
            )
            lgacc = cxl[:, 2 * H * (Dh + 1):]

            masks.make_identity(nc, ident[:])
            nc.sync.dma_start(out=wqkvT_sb[:], in_=wqkvT_d[:])
            nc.sync.dma_start(out=bqk_sb[:], in_=bqk_d[:])
            for p in range(4):
                nc.sync.dma_start(out=xT_sb[:, 2 * p * S:(2 * p + 2) * S],
                                  in_=xT_d[:, 2 * p * S:(2 * p + 2) * S])
            nc.sync.dma_start(out=wfT_sb[:], in_=wfT_d[:])
            nc.sync.dma_start(out=c1b_sb[:], in_=c1b_d[:])
            nc.sync.dma_start(out=w2a_sb[:], in_=w2a_d[:])
            nc.vector.memset(v_sb[:, :, :, :, :, Dh:Dh + 1], 1.0)
            nc.vector.memset(h1a_sb[H2:H2 + 1, :, :], 1.0)

            def proj_tile(p, ev, qk):
                # q/k projection for token chunks 2p, 2p+1, one (event, q|k)
                ps = stp.tile([D, 2, S], F32, name="qkps", tag="st")
                for c in range(2):
                    n = 2 * p + c
                    nc.tensor.matmul(
                        ps[:, c, :],
                        wqkvT_sb[:, ev, qk, :],
                        xT_sb[:, n * S:(n + 1) * S],
                    )
                dst = qT_sb if qk == 0 else kT_sb
                nc.scalar.activation(
                    dst[:, ev, 2 * p * S:(2 * p + 2) * S],
                    ps[:],
                    AF.Identity,
                    bias=bqk_sb[:, ev, qk:qk + 1],
                )

            def proj_pair(p):
                for ev in range(EV):
                    for qk in range(2):
                        proj_tile(p, ev, qk)

            def project_v_half(b, half):
                psv = gpp.tile([D, S], F32, name="vps", tag="gp")
                for c2 in range(2):
                    tch = 4 * b + 2 * half + c2
                    nc.tensor.matmul(
                        psv[:, c2 * 256:(c2 + 1) * 256],
                        xT_sb[:, tch * D:(tch + 1) * D],
                        wqkvT_sb[:, :, 2, :],
                    )
                for ev2 in range(EV):
                    nc.vector.tensor_copy(
                        v_sb[:, ev2, b, 2 * half:2 * half + 2, :, 0:Dh],
                        psv[:].rearrange(
                            "p (c e h d) -> p c e h d", c=2, e=2, h=H
                        )[:, :, ev2],
                    )

            proj_pair(0)
            pend_proj = [(p, ev, qk) for p in (1, 2, 3)
                         for ev in range(EV) for qk in range(2)]

            # software pipeline, lagged stages, interleaved so PE always
            # has PV/Wf/W2 work to run while it waits for score-ring slots:
            #   iter i: scores+exp(i) interleaved with
            #           PV+norm+ctxT(i-1), Wf+relu(i-2), W2(i-3), proj
            pend_pv = None    # (ev, b, eb, pt)
            pend_wf = None    # (ev, eb, ctxT) fresh from PV stage
            pend_wf2 = None   # (ev, eb, ctxT) aged one iteration (dma slack)
            pend_w2 = None    # (ev, eb)

            def emit_w2(pev, peb):
                for ic in range(4):
                    col = peb * 4 + ic
                    nc.tensor.matmul(
                        lgacc[:, col:col + 1],
                        h1a_sb[:, peb % 2, ic * D:(ic + 1) * D],
                        w2a_sb[:, pev:pev + 1],
                    )

            def emit_wf_relu(pev, peb, pctxT):
                gp = gpp.tile([H2, S], F32, name="gp", tag="gp")
                nc.tensor.matmul(gp[:], wfT_sb[:, pev, :], pctxT[:])
                nc.vector.tensor_scalar(
                    h1a_sb[0:H2, peb % 2, :],
                    gp[:],
                    c1b_sb[0:H2, pev:pev + 1],
                    0.0,
                    ALU.add,
                    ALU.max,
                )

            def emit_pv_quarter(pev, pb, ppt, icw, ic2):
                ctx = ctxps
                ic = 2 * icw + ic2
                for h in range(H):
                    for jc in range(4):
                        nc.tensor.matmul(
                            ctx[:, ic2, h, :],
                            ppt[:, jc, h, ic * D:(ic + 1) * D],
                            v_sb[:, pev, pb, jc, h, :],
                            start=(jc == 0),
                            stop=(jc == 3),
                        )

            def emit_pv_norm(ctx_pair, icw):
                ctxn, linv = ctx_pair
                ctx = ctxps
                nc.vector.reciprocal(linv[:, icw, :, :], ctx[:, :, :, Dh])
                nc.vector.tensor_tensor(
                    ctxn[:, 2 * icw:2 * icw + 2, :, :],
                    ctx[:, :, :, 0:Dh],
                    linv[:, icw, :, :, None].to_broadcast([D, 2, H, Dh]),
                    ALU.mult,
                )

            def emit_pv_half(pev, pb, ppt, ctx_pair, icw):
                emit_pv_quarter(pev, pb, ppt, icw, 0)
                emit_pv_quarter(pev, pb, ppt, icw, 1)
                emit_pv_norm(ctx_pair, icw)

            seq = [(ev, b) for ev in range(EV) for b in range(B)]
            for ev, b in seq:
                eb = ev * B + b
                t0 = b * S

                pv_pair = None
                if pend_pv is not None:
                    pv_pair = (
                        work.tile([D, 4, H, Dh], BF16, name="ctxn", tag="cn"),
                        work.tile([D, 2, 2, H], F32, name="linv", tag="li"),
                    )

                pt = work.tile([D, 4, H, S], BF16, name="pt", tag="pt")

                def qk_chunk(jc, hp):
                    st = stp.tile([D, 2, S], F32, name="st", tag="st")
                    for hh in range(2):
                        h = 2 * hp + hh
                        nc.tensor.matmul(
                            st[:, hh, :],
                            kT_sb[32 * h:32 * h + 32, ev,
                                  t0 + jc * D:t0 + (jc + 1) * D],
                            qT_sb[32 * h:32 * h + 32, ev, t0:t0 + S],
                            tile_position=(32 * h, 0),
                        )
                    if is_act_chunk(eb, jc * 2 + hp):
                        nc.scalar.activation(
                            pt[:, jc, 2 * hp:2 * hp + 2, :], st[:], AF.Exp
                        )
                    else:
                        nc.vector.tensor_scalar(
                            pt[:, jc, 2 * hp:2 * hp + 2, :].bitcast(I16),
                            st[:],
                            SCH_SCALE,
                            SCH_BASE,
                            ALU.mult,
                            ALU.add,
                        )

                def next_proj():
                    # 12 pending q/k proj tiles spread over ev-0 iterations:
                    # 2 per iteration keeps them ~2 batches ahead of use
                    if ev == 0 and pend_proj:
                        proj_tile(*pend_proj.pop(0))

                last = (eb == EV * B - 1)
                if last and pend_pv is not None:
                    # final iteration: PV(14) runs before the ring-gated QK
                    # chunks so its chain doesn't land in the drain tail
                    emit_pv_half(pend_pv[0], pend_pv[1], pend_pv[3], pv_pair, 0)
                    emit_pv_half(pend_pv[0], pend_pv[1], pend_pv[3], pv_pair, 1)
                qk_chunk(0, 0)
                qk_chunk(0, 1)
                next_proj()
                if pend_w2 is not None:
                    emit_w2(*pend_w2)
                    pend_w2 = None
                qk_chunk(1, 0)
                if not last and pend_pv is not None:
                    emit_pv_quarter(pend_pv[0], pend_pv[1], pend_pv[3], 0, 0)
                qk_chunk(1, 1)
                if pend_wf2 is not None:
                    emit_wf_relu(*pend_wf2)
                    pend_w2 = (pend_wf2[0], pend_wf2[1])
                    pend_wf2 = None
                if pend_wf is not None:
                    pend_wf2 = pend_wf
                    pend_wf = None
                if not last and pend_pv is not None:
                    emit_pv_quarter(pend_pv[0], pend_pv[1], pend_pv[3], 0, 1)
                    emit_pv_norm(pv_pair, 0)
                qk_chunk(2, 0)
                if not last and pend_pv is not None:
                    emit_pv_quarter(pend_pv[0], pend_pv[1], pend_pv[3], 1, 0)
                if ev == 0:
                    project_v_half(b, 0)
                qk_chunk(2, 1)
                if not last and pend_pv is not None:
                    emit_pv_quarter(pend_pv[0], pend_pv[1], pend_pv[3], 1, 1)
                    emit_pv_norm(pv_pair, 1)
                next_proj()
                qk_chunk(3, 0)
                if ev == 0:
                    project_v_half(b, 1)
                qk_chunk(3, 1)
                if pend_pv is not None:
                    ctxT = work.tile([D, S], BF16, name="ctxT", tag="ct", bufs=3)
                    for ic in range(4):
                        nc.sync.dma_start(
                            out=ctxT[:, ic * D:(ic + 1) * D],
                            in_=pv_pair[0][:, ic, :, :],
                            transpose=True,
                        )
                    pend_wf = (pend_pv[0], pend_pv[2], ctxT)
                    pend_pv = None
                pend_pv = (ev, b, eb, pt)

            # drain the pipeline tail
            for _ in range(4):
                if pend_w2 is not None:
                    emit_w2(*pend_w2)
                    pend_w2 = None
                if pend_wf2 is not None:
                    emit_wf_relu(*pend_wf2)
                    pend_w2 = (pend_wf2[0], pend_wf2[1])
                    pend_wf2 = None
                if pend_wf is not None:
                    pend_wf2 = pend_wf
                    pend_wf = None
                if pend_pv is not None:
                    pev, pb, peb, ppt = pend_pv
                    pv_pair = (
                        work.tile([D, 4, H, Dh], BF16, name="ctxn", tag="cn"),
                        work.tile([D, 2, 2, H], F32, name="linv", tag="li"),
                    )
                    emit_pv_half(pev, pb, ppt, pv_pair, 0)
                    emit_pv_half(pev, pb, ppt, pv_pair, 1)
                    ctd = stp.tile([D, 2, S], F32, name="ctd", tag="st")
                    ct_bf = ctd[:, 0, 0:256].bitcast(BF16)
                    for ic in range(4):
                        nc.tensor.transpose(
                            ct_bf[:, ic * D:(ic + 1) * D],
                            pv_pair[0][:, ic, :, :], ident[:]
                        )
                    ctxT = work.tile([D, S], BF16, name="ctxT", tag="ct", bufs=3)
                    nc.vector.tensor_copy(ctxT[:], ct_bf[:])
                    pend_wf = (pev, peb, ctxT)
                    pend_pv = None

            # gather logits: [128 i, 64 (eb,ic)] -> transpose -> [64, 128]
            nc.vector.tensor_copy(lg_sb[:], lgacc[:])
            lgt = gpp.tile([H2, 64], F32, name="lgt", tag="gp")
            nc.tensor.transpose(lgt[:, :].bitcast(BF16), lg_sb[:], ident[:])
            nc.vector.tensor_copy(lgT_sb[:], lgt[:, :].bitcast(BF16))
            nc.sync.dma_start(
                out=out_d[:].rearrange("e b (c i) -> (e b c) i", c=4),
                in_=lgT_sb[:],
            )

    nc.compile()
    return nc


def _prep_inputs(lstm_features, Wqkv, bqkv, Wo, bo, W1, b1, W2, b2):
    """Host-side per-core input prep (numpy, fp32 -> bf16 where PE-facing)."""
    bf = ml_dtypes.bfloat16
    x = np.asarray(lstm_features, np.float32).reshape(T, D)
    xT = np.ascontiguousarray(x.T).astype(bf)
    scale = 1.0 / np.sqrt(np.float32(Dh))

    in_maps = []
    for c in range(NCORES):
        evs = [2 * c, 2 * c + 1]
        wqkvT = np.zeros((D, EV, 3, D), np.float32)
        bqk = np.zeros((D, EV, 2), np.float32)
        wfT = np.zeros((D, EV, H2), np.float32)
        c1b = np.zeros((H2, EV), np.float32)
        w2a = np.zeros((H2 + 1, EV), np.float32)
        for i, e in enumerate(evs):
            Wq = Wqkv[e, 0:D, :] * scale
            Wk = Wqkv[e, D:2 * D, :]
            Wv = Wqkv[e, 2 * D:3 * D, :]
            wqkvT[:, i, 0, :] = Wq.T
            wqkvT[:, i, 1, :] = Wk.T
            wqkvT[:, i, 2, :] = Wv.T
            bqk[:, i, 0] = bqkv[e, 0:D] * scale
            bqk[:, i, 1] = bqkv[e, D:2 * D]
            bv = bqkv[e, 2 * D:3 * D]
            bo_eff = Wo[e] @ bv + bo[e]
            Wf = W1[e] @ Wo[e]          # [H2, D]
            wfT[:, i, :] = Wf.T
            c1b[:, i] = W1[e] @ bo_eff + b1[e]
            w2a[0:H2, i] = W2[e, 0, :]
            w2a[H2, i] = b2[e, 0]
        in_maps.append({
            "xT": xT,
            "wqkvT": wqkvT.astype(bf),
            "bqk": bqk,
            "wfT": wfT.astype(bf),
            "c1b": c1b,
            "w2a": w2a.astype(bf),
        })
    return in_maps


def kernel(lstm_features, Wqkv, bqkv, Wo, bo, W1, b1, W2, b2, _trace=False):
    global _CACHED_NC
    args = [np.asarray(a, np.float32) for a in
            (lstm_features, Wqkv, bqkv, Wo, bo, W1, b1, W2, b2)]
    in_maps = _prep_inputs(*args)
    if _CACHED_NC is None:
        _CACHED_NC = build_nc()
    res = run_bass_kernel_spmd(
        _CACHED_NC, in_maps, list(range(NCORES)), trace=_trace
    )
    logits = np.concatenate(
        [np.asarray(res.results[c]["out"], np.float32) for c in range(NCORES)],
        axis=0,
    )  # [16, 8, 512]
    out = np.ascontiguousarray(logits.transpose(1, 2, 0))  # [B, S, E]
    if _trace:
        return out, res
    return out
